# revision 76
# baseline (speedup 1.0000x reference)
"""Trainium2 Bass kernel for the CapacityNN PINN forward pass.

Computes, for N = B*S collocation points x = (s, t):
  U   = MLP([s_norm, t_norm]) * tgt_std + tgt_mean
  F   = U_t  - G(U)             (G = Verhulst logistic growth term)
  F_t = U_tt - G'(U) * U_t
where U_t/U_tt are 1st/2nd derivatives w.r.t. t_norm, computed exactly by
second-order forward-mode Taylor (jet) propagation through the tanh MLP.

Sharding: pure data parallel over 8 NeuronCores (8192 points/core),
MLP weights + PDE scalars replicated. All math runs on-device; the host
only folds weights (O(H^2)) and reorders data (transpose, shard/gather).

Device layout: hidden dim (256) on partitions (2 halves m of 128), points
on the free dim in chunks of 1024; the per-layer stream tiles hold both
halves side by side as [128, 2048] f16 so each elementwise op covers the
whole layer. All matmuls are f16 with f32 PSUM accumulation (1 col/cycle,
same rate as f32r, half the error-prone-ness of the elementwise budget).
Streams: Hv = tanh values; H1 carries sqrt(2)*a1 (sqrt2 folded into the
layer-1 weight copy so st = zc1^2 = 2*z1^2 needs no extra scale; the
final projection's H1 column of W4 is divided by sqrt2); H2 = a2.

Per hidden layer l and chunk (both halves):
  PE : 24 matmuls [128,128]@[128,512] -> pzv/pz1/pz2 ([128,1024] psum)
  Act: tanh(pzv)->av, copy(pz1)->zc1, copy(pz2)->qtc (only Act+DVE may
       touch PSUM on TRN2; GPSIMD/DMA cannot)
  DVE: ee=av*av (l1,l2 on Pool), dm=1-ee (in-place, 4x-mode ts),
       h1=dm*zc1, st=zc1*zc1, tt=av*st, qt=qtc-tt (in-place), h2=dm*qt
       (f16 SBUF ops run at DVE 2x rate, 0.52 ns/col)
  Pool: ee for l1/l2 + half of st(l2) (slow 2 ns/col engine, overflow only)

Schedule: all 8 chunks software-pipelined flat (in-order engine queues
are the schedule): L0 x8, L1 x8, L2 x8, then L3 with the final
projections trailing 4 chunks behind; the PDE tail runs in two halves,
the first overlapped with the second half's compute. PSUM is one
4-deep pool of [128,1024] tiles (8 banks); the final projection
writes rows 0:3 of a pool slot. Weight/scalar loads are batched into
~13 wide DMAs split across the SP and Act queues so the first matmul
starts at ~5 us.
"""

import os
import sys
import tempfile

import numpy as np

for _p in ("/opt/trn_rl_repo", "/root/.axon_site/_ro/trn_rl_repo"):
    if os.path.isdir(_p) and _p not in sys.path:
        sys.path.insert(0, _p)

import concourse.bass as bass
import concourse.bacc as bacc
import concourse.tile as tile
from concourse import mybir
from concourse.bass_utils import run_bass_kernel_spmd

AF = mybir.ActivationFunctionType
OP = mybir.AluOpType
F32 = mybir.dt.float32
F32R = mybir.dt.float32r
F16 = mybir.dt.float16

NCORES = 8
B, S, H = 512, 128, 256
N = B * S                  # 65536 points
NLOC = N // NCORES         # 8192 points per core
CH = 1024                  # points per on-chip chunk
NCHUNK = NLOC // CH
PT = CH // 512             # 512-wide point tiles per chunk
PPP = NLOC // 128          # points per partition in the tail layout (64)
SQRT2 = float(np.sqrt(2.0))


def _build():
    nc = bacc.Bacc(
        "TRN2",
        target_bir_lowering=False,
        debug=False,
        enable_asserts=False,
        num_devices=NCORES,
    )

    def din(name, shape, dt=F32):
        return nc.dram_tensor(name, list(shape), dt, kind="ExternalInput").ap()

    x2 = din("x2", (2, NLOC), F32R)            # rows: raw s, raw t (per-core slice)
    w0t = din("w0t", (2, H), F32R)             # W0.T
    wth = {l: din(f"w{l}th", (H, H), F16) for l in (1, 2, 3)}  # fp16 Wl.T
    w1wt = din("w1wt", (H, H), F16)     # sqrt2*(W1*diag(w0c1)).T fp16
    w1w2t = din("w1w2t", (H, H), F16)   # (-2*W1*diag(w0c1^2)).T fp16
    lt4h = din("lt4h", (6, 128, 3), F16)   # block-diag final lhsT (scaled)
    ball = din("ball", (H, 4))          # biases b0..b3 as columns
    aux = din("aux", (10,))             # packed scalars (see _prep_in_maps)
    in_std = din("in_std", (2,))
    out = nc.dram_tensor("out", [3, NLOC], F32, kind="ExternalOutput").ap()

    with tile.TileContext(nc) as tc:
        from contextlib import ExitStack

        with ExitStack() as ctx:
            const = ctx.enter_context(tc.tile_pool(name="const", bufs=1))
            sb = ctx.enter_context(tc.tile_pool(name="sb", bufs=1))
            ps = ctx.enter_context(tc.tile_pool(name="ps", bufs=1, space="PSUM"))

            # ---------- one-time prep (DMAs batched; L0-critical first) ------
            inv01 = const.tile([2, 1], F32, name="inv01")
            nc.sync.dma_start(
                out=inv01, in_=bass.AP(in_std.tensor, 0, [[1, 2], [1, 1]])
            )
            nc.vector.tensor_scalar(inv01, inv01, 1e-8, None, OP.add)
            nc.vector.reciprocal(inv01, inv01)
            w0ts = const.tile([2, H], F32R, name="w0ts")            # rows x 1/std
            nc.sync.dma_start(out=w0ts, in_=w0t)
            nc.vector.tensor_scalar(w0ts, w0ts, inv01, None, OP.mult)

            # aux = [m0, m1, s0, s1, lgr, lcc, lil, tgt_mean, tgt_std, b4]
            auxt = const.tile([128, 10], F32, name="auxt")
            nc.sync.dma_start(
                out=auxt, in_=bass.AP(aux.tensor, 0, [[0, 128], [1, 10]])
            )
            bc_m0 = auxt[:, 0:1]
            bc_m1 = auxt[:, 1:2]
            bc_s0 = auxt[:, 2:3]
            bc_s1 = auxt[:, 3:4]
            bc_lgr = auxt[:, 4:5]
            bc_lcc = auxt[:, 5:6]
            bc_lil = auxt[:, 6:7]
            bc_tm = auxt[:, 7:8]
            bc_ts = auxt[:, 8:9]
            bc_b4 = auxt[:, 9:10]

            def new1(name):
                return const.tile([128, 1], F32, name=name)

            inv0 = new1("inv0")
            nc.vector.tensor_scalar(inv0, bc_s0, 1e-8, None, OP.add)
            nc.vector.reciprocal(inv0, inv0)
            inv1 = new1("inv1")
            nc.vector.tensor_scalar(inv1, bc_s1, 1e-8, None, OP.add)
            nc.vector.reciprocal(inv1, inv1)

            r_t = new1("r_t")
            nc.scalar.activation(r_t, bc_lgr, AF.Exp, 0.0, -1.0)   # exp(-lgr)
            K_t = new1("K_t")
            nc.scalar.activation(K_t, bc_lcc, AF.Sigmoid)
            nc.vector.tensor_scalar(K_t, K_t, 0.8, 0.2, OP.mult, OP.add)
            C_t = new1("C_t")
            nc.scalar.activation(C_t, bc_lil, AF.Sigmoid)
            nc.vector.tensor_scalar(C_t, C_t, 0.1, None, OP.mult)
            ikc = new1("ikc")                                      # 1/(K-C)
            nc.vector.tensor_tensor(ikc, K_t, C_t, OP.subtract)
            nc.vector.reciprocal(ikc, ikc)
            nr = new1("nr")                                        # -r
            nc.vector.tensor_scalar(nr, r_t, -1.0, None, OP.mult)
            c1 = new1("c1")                                        # -1/(K-C)
            nc.vector.tensor_scalar(c1, ikc, -1.0, None, OP.mult)
            mc3 = new1("mc3")                                      # 2r/(K-C)
            nc.vector.tensor_tensor(mc3, r_t, ikc, OP.mult)
            nc.vector.tensor_scalar(mc3, mc3, 2.0, None, OP.mult)
            tmb = new1("tmb")                                      # b4*ts + tm
            nc.vector.tensor_tensor(tmb, bc_b4, bc_ts, OP.mult)
            nc.vector.tensor_tensor(tmb, tmb, bc_tm, OP.add)
            m0i = new1("m0i")                                      # m0/(s0+eps)
            nc.vector.tensor_tensor(m0i, bc_m0, inv0, OP.mult)
            m1i = new1("m1i")
            nc.vector.tensor_tensor(m1i, bc_m1, inv1, OP.mult)

            # biases: [128, 4] per m-half (cols = layers 0..3)
            ballt = []
            for m in range(2):
                t = const.tile([128, 4], F32, name=f"ball_{m}")
                nc.sync.dma_start(
                    out=t, in_=bass.AP(ball.tensor, 128 * m * 4, [[4, 128], [1, 4]])
                )
                ballt.append(t)
            bl = {l: [ballt[m][:, l : l + 1] for m in range(2)] for l in (1, 2, 3)}

            beta0 = []
            for m in range(2):
                a = const.tile([128, 2], F32R, name=f"w0c_{m}")     # W0 rows [128m:128m+128]
                nc.sync.dma_start(
                    out=a, in_=bass.AP(w0t.tensor, 128 * m, [[1, 128], [H, 2]])
                )
                u1 = new1(f"u1_{m}")
                nc.vector.tensor_tensor(u1, a[:, 0:1], m0i, OP.mult)
                u2 = new1(f"u2_{m}")
                nc.vector.tensor_tensor(u2, a[:, 1:2], m1i, OP.mult)
                nc.vector.tensor_tensor(u1, u1, u2, OP.add)
                bet = new1(f"beta_{m}")                            # b0 - u1
                nc.vector.scalar_tensor_tensor(
                    bet, u1, -1.0, ballt[m][:, 0:1], OP.mult, OP.add
                )
                beta0.append(bet)

            # ---------- hidden-layer weights (f16, [128,256] per k-half) -----
            def load_w(srcd, nm):
                halves = []
                for kk in range(2):
                    t = const.tile([128, H], F16, name=f"{nm}_{kk}")
                    nc.sync.dma_start(
                        out=t,
                        in_=bass.AP(srcd.tensor, kk * 128 * H, [[H, 128], [1, H]]),
                    )
                    halves.append(t)
                return [
                    [halves[kk][:, mm * 128 : (mm + 1) * 128] for mm in range(2)]
                    for kk in range(2)
                ]

            wtw = load_w(w1wt, "wtw")
            wtw2 = load_w(w1w2t, "wtw2")
            wt16 = {1: load_w(wth[1], "wth1")}

            # final-projection block-diag lhsT: one [128, 6x3] load
            lt4t = const.tile([128, 6, 3], F16, name="lt4t")
            nc.sync.dma_start(
                out=lt4t, in_=bass.AP(lt4h.tensor, 0, [[3, 128], [384, 6], [1, 3]])
            )
            lt4 = [
                [lt4t[:, 2 * s_idx + kk, :] for kk in range(2)] for s_idx in range(3)
            ]

            # the rest of the weights stream in behind the first chunks' L0/L1
            for l in (2, 3):
                wt16[l] = load_w(wth[l], f"wth{l}")

            # ---------- main loop over point chunks ----------
            # Two chunks are processed in lock-step (software pipelining):
            # stage order L0(c0) L0(c1) L1(c0) L1(c1) ... FIN(c0) FIN(c1), so
            # each in-order engine queue works on chunk c0's stage while the
            # producers of chunk c1's stage run on other engines.
            y3f = sb.tile([3, NLOC], F16, name="y3f")
            state = {}   # chunk -> (Hv, H1, H2)

            # streams/transients hold both 128-row halves side by side:
            # [128, 2*CH], half m occupying cols [m*CH : (m+1)*CH].
            def emit_l0(c):
                p = c % 8
                q = c % 4
                x2c = sb.tile([2, CH], F32R, tag=f"x2c{c % 2}", bufs=2)
                dma_eng = nc.scalar if c < 4 else nc.sync
                dma_eng.dma_start(out=x2c, in_=x2[:, c * CH : (c + 1) * CH])
                av = sb.tile([128, 2 * CH], F16, tag=f"hv{p}", bufs=1, name="av0")
                for m in range(2):
                    pz0 = ps.tile([128, CH], F32, tag="pz", bufs=4, name="pz0")
                    for i in range(PT):
                        nc.tensor.matmul(
                            pz0[:, i * 512 : (i + 1) * 512],
                            w0ts[:, m * 128 : (m + 1) * 128],
                            x2c[:, i * 512 : (i + 1) * 512],
                            start=True,
                            stop=True,
                        )
                    nc.scalar.activation(
                        av[:, m * CH : (m + 1) * CH], pz0, AF.Tanh, beta0[m]
                    )
                ee = sb.tile([128, 2 * CH], F16, tag=f"ee{q}", bufs=1, name="ee0")
                nc.vector.tensor_tensor(ee, av, av, OP.mult)
                dm = sb.tile([128, 2 * CH], F16, tag=f"h1_{p}", bufs=1, name="dm0")
                nc.vector.tensor_scalar(dm, ee, -1.0, 1.0, OP.mult, OP.add)
                ad = sb.tile([128, 2 * CH], F16, tag=f"h2_{p}", bufs=1, name="ad0")
                nc.vector.tensor_tensor(ad, av, dm, OP.mult)
                state[c] = (av, dm, ad)

            def emit_layer(c, l):
                p = c % 8
                q = c % 4
                Hv, H1, H2 = state[c]
                w_v = wt16[l]
                w_1 = wtw if l == 1 else wt16[l]
                w_2 = wtw2 if l == 1 else wt16[l]
                av = sb.tile([128, 2 * CH], F16, tag=f"hv{p}", bufs=1, name="av")
                zc1 = sb.tile([128, 2 * CH], F16, tag=f"zc1{q}", bufs=1, name="zc1")
                qtc = sb.tile([128, 2 * CH], F16, tag=f"zc2{q}", bufs=1, name="qtc")
                for m in range(2):
                    # --- primal + first-tangent matmuls, Act psum readers ---
                    pzv = ps.tile([128, CH], F32, tag="pz", bufs=4, name="pzv")
                    for i in range(PT):
                        for kk in range(2):
                            nc.tensor.matmul(
                                pzv[:, i * 512 : (i + 1) * 512],
                                w_v[kk][m],
                                Hv[:, kk * CH + i * 512 : kk * CH + (i + 1) * 512],
                                start=(kk == 0),
                                stop=(kk == 1),
                            )
                    nc.scalar.activation(
                        av[:, m * CH : (m + 1) * CH], pzv, AF.Tanh, bl[l][m]
                    )

                    pz1 = ps.tile([128, CH], F32, tag="pz", bufs=4, name="pz1")
                    for i in range(PT):
                        for kk in range(2):
                            nc.tensor.matmul(
                                pz1[:, i * 512 : (i + 1) * 512],
                                w_1[kk][m],
                                H1[:, kk * CH + i * 512 : kk * CH + (i + 1) * 512],
                                start=(kk == 0),
                                stop=(kk == 1),
                            )
                    nc.scalar.copy(zc1[:, m * CH : (m + 1) * CH], pz1)  # sqrt2*z1

                # --- second-tangent matmuls ---
                for m in range(2):
                    pz2 = ps.tile([128, CH], F32, tag="pz", bufs=4, name="pz2")
                    for i in range(PT):
                        for kk in range(2):
                            nc.tensor.matmul(
                                pz2[:, i * 512 : (i + 1) * 512],
                                w_2[kk][m],
                                H2[:, kk * CH + i * 512 : kk * CH + (i + 1) * 512],
                                start=(kk == 0),
                                stop=(kk == 1),
                            )
                    nc.scalar.copy(qtc[:, m * CH : (m + 1) * CH], pz2)  # z2

                # --- f16 SBUF jet algebra, both halves per op ---
                ee = sb.tile([128, 2 * CH], F16, tag=f"ee{q}", bufs=1, name="ee")
                if l in (1, 2):
                    nc.gpsimd.tensor_tensor(ee, av, av, OP.mult)
                else:
                    nc.vector.tensor_tensor(ee, av, av, OP.mult)
                dm = ee  # in-place: dm = 1 - ee
                nc.vector.tensor_scalar(dm, ee, -1.0, 1.0, OP.mult, OP.add)
                h1t = sb.tile([128, 2 * CH], F16, tag=f"h1_{p}", bufs=1, name="h1t")
                nc.vector.tensor_tensor(h1t, dm, zc1, OP.mult)
                st = sb.tile([128, 2 * CH], F16, tag=f"st{q}", bufs=1, name="st")
                if l == 2:
                    # split halves across DVE/Pool to shed DVE columns
                    nc.vector.tensor_tensor(
                        st[:, 0:CH], zc1[:, 0:CH], zc1[:, 0:CH], OP.mult
                    )
                    nc.gpsimd.tensor_tensor(
                        st[:, CH:], zc1[:, CH:], zc1[:, CH:], OP.mult
                    )
                else:
                    nc.vector.tensor_tensor(st, zc1, zc1, OP.mult)  # 2*z1^2
                ttp = st  # in-place: 2a*z1^2
                nc.vector.tensor_tensor(ttp, av, st, OP.mult)
                qt = qtc  # in-place: z2 - 2a*z1^2
                nc.vector.tensor_tensor(qt, qtc, ttp, OP.subtract)
                h2t = sb.tile([128, 2 * CH], F16, tag=f"h2_{p}", bufs=1, name="h2t")
                nc.vector.tensor_tensor(h2t, dm, qt, OP.mult)
                state[c] = (av, h1t, h2t)

            def emit_fin(c):
                Hv, H1, H2 = state[c]
                py = ps.tile([128, CH], F32, tag="pz", bufs=4, name="py")
                for i in range(PT):
                    first = True
                    for s_idx, stream in enumerate((Hv, H1, H2)):
                        for kk in range(2):
                            nc.tensor.matmul(
                                py[0:3, i * 512 : (i + 1) * 512],
                                lt4[s_idx][kk],
                                stream[:, kk * CH + i * 512 : kk * CH + (i + 1) * 512],
                                start=first,
                                stop=(s_idx == 2 and kk == 1),
                            )
                            first = False
                nc.scalar.copy(y3f[:, c * CH : (c + 1) * CH], py[0:3, :])

            NH = NLOC // 2
            PPH = NH // 128

            def emit_tail(h):
                # PDE algebra for half h of the points (reshaped to [128, PPH])
                tp = sb.tile([128, 3 * PPH], F16, tag=f"tp{h % 2}", bufs=2, name="tp")
                dma_q = nc.sync if h < 1 else nc.scalar
                for s_idx in range(3):
                    dma_q.dma_start(
                        out=tp[:, s_idx * PPH : (s_idx + 1) * PPH],
                        in_=y3f[s_idx : s_idx + 1, h * NH : (h + 1) * NH],
                    )
                yv = tp[:, 0:PPH]
                yt = tp[:, PPH : 2 * PPH]
                ytt = tp[:, 2 * PPH : 3 * PPH]
                oc = sb.tile([128, 3 * PPH], F32, tag=f"oc{h % 2}", bufs=2, name="oc")
                U = oc[:, 0:PPH]
                Fo = oc[:, PPH : 2 * PPH]
                Ft = oc[:, 2 * PPH : 3 * PPH]

                def tl(nm):
                    return sb.tile([128, PPH], F32, tag=f"{nm}{h % 2}", bufs=2, name=nm)

                ut = tl("ut")
                utt = tl("utt")
                vv = tl("vv")
                v2 = tl("v2")
                w1 = tl("w1")
                q1 = tl("q1")
                t1 = tl("t1")
                nc.vector.tensor_scalar(U, yv, bc_ts, tmb, OP.mult, OP.add)
                nc.vector.tensor_scalar(ut, yt, bc_ts, None, OP.mult)
                nc.vector.tensor_scalar(utt, ytt, bc_ts, None, OP.mult)
                nc.vector.tensor_scalar(vv, U, C_t, None, OP.subtract)
                nc.gpsimd.tensor_tensor(v2, vv, vv, OP.mult)
                nc.vector.scalar_tensor_tensor(w1, v2, c1, vv, OP.mult, OP.add)
                nc.vector.scalar_tensor_tensor(Fo, w1, nr, ut, OP.mult, OP.add)
                nc.gpsimd.tensor_tensor(q1, vv, ut, OP.mult)
                nc.vector.scalar_tensor_tensor(t1, ut, nr, utt, OP.mult, OP.add)
                nc.vector.scalar_tensor_tensor(Ft, q1, mc3, t1, OP.mult, OP.add)
                for s_idx in range(3):
                    dma_q.dma_start(
                        out=out[s_idx : s_idx + 1, h * NH : (h + 1) * NH],
                        in_=oc[:, s_idx * PPH : (s_idx + 1) * PPH],
                    )

            for c in range(NCHUNK):
                emit_l0(c)
            for l in (1, 2):
                for c in range(NCHUNK):
                    emit_layer(c, l)
            pend = []
            for c in range(NCHUNK):
                emit_layer(c, 3)
                pend.append(c)
                if len(pend) > 4:
                    fc = pend.pop(0)
                    emit_fin(fc)
                    if fc == 3:
                        emit_tail(0)
            for fc in pend:
                emit_fin(fc)
                if fc == 3:
                    emit_tail(0)
            emit_tail(1)


    nc.compile()
    return nc


_STATE = {}


def _get_nc():
    if "nc" not in _STATE:
        _STATE["nc"] = _build()
    return _STATE["nc"]


def _make_lt4(w4):
    scales = (1.0, 1.0 / SQRT2, 1.0)
    out = np.zeros((6, 128, 3), np.float32)
    for s_idx in range(3):
        for kk in range(2):
            out[2 * s_idx + kk, :, s_idx] = (
                w4[0, kk * 128 : (kk + 1) * 128] * scales[s_idx]
            )
    return out


def _prep_in_maps(inputs):
    f = np.float32

    def arr(k):
        return np.ascontiguousarray(np.asarray(inputs[k], f))

    x = np.asarray(inputs["inputs"], f).reshape(N, 2)
    aux = np.concatenate([
        arr("in_mean"), arr("in_std"),
        arr("log_growth_rate").reshape(1),
        arr("log_carrying_capacity").reshape(1),
        arr("log_initial_loss").reshape(1),
        arr("tgt_mean"), arr("tgt_std"), arr("b4").reshape(1),
    ]).astype(f)
    ball = np.stack([arr(f"b{l}") for l in range(4)], axis=1)
    shared = {
        "w0t": np.ascontiguousarray(arr("W0").T),
        "lt4h": _make_lt4(arr("W4").reshape(1, H)).astype(np.float16),
        "w1th": np.ascontiguousarray(arr("W1").T).astype(np.float16),
        "w1wt": np.ascontiguousarray(
            (SQRT2 * arr("W1") * arr("W0")[:, 1][None, :]).T
        ).astype(np.float16),
        "w1w2t": np.ascontiguousarray(
            (arr("W1") * (-2.0 * arr("W0")[:, 1] ** 2)[None, :]).T
        ).astype(np.float16),
        "w2th": np.ascontiguousarray(arr("W2").T).astype(np.float16),
        "w3th": np.ascontiguousarray(arr("W3").T).astype(np.float16),
        "ball": np.ascontiguousarray(ball),
        "aux": aux,
        "in_std": arr("in_std"),
    }
    in_maps = []
    for c in range(NCORES):
        m = dict(shared)
        m["x2"] = np.ascontiguousarray(x[c * NLOC : (c + 1) * NLOC].T)
        in_maps.append(m)
    return in_maps


def run(inputs, trace=False):
    nc = _get_nc()
    in_maps = _prep_in_maps(inputs)
    kw = {}
    if trace:
        kw["tmpdir"] = tempfile.mkdtemp(prefix="bassk_prof_")
    res = run_bass_kernel_spmd(
        nc, in_maps, core_ids=list(range(NCORES)), trace=trace, **kw
    )
    U = np.empty((N,), np.float32)
    F = np.empty((N,), np.float32)
    Ft = np.empty((N,), np.float32)
    for c in range(NCORES):
        o = res.results[c]["out"]
        U[c * NLOC : (c + 1) * NLOC] = o[0]
        F[c * NLOC : (c + 1) * NLOC] = o[1]
        Ft[c * NLOC : (c + 1) * NLOC] = o[2]
    shp = (B, S, 1)
    return (U.reshape(shp), F.reshape(shp), Ft.reshape(shp)), res


def kernel(**inputs):
    outs, _ = run(inputs, trace=False)
    return outs


# ---------------------------------------------------------------------------
# Dev-loop timing: persistent jitted executable (mirrors
# bass2jax.run_bass_via_pjrt's multi-core branch) so repeated executions
# reuse one compiled NEFF and can be timed back-to-back.
# ---------------------------------------------------------------------------
def _make_runner():
    if "runner" in _STATE:
        return _STATE["runner"]
    import jax
    from jax.experimental.shard_map import shard_map
    from jax.sharding import Mesh, PartitionSpec
    from concourse import bass2jax

    bass2jax.install_neuronx_cc_hook()
    nc = _get_nc()

    in_names, out_names, out_avals, zero_outs = [], [], [], []
    for alloc in nc.m.functions[0].allocations:
        if not isinstance(alloc, mybir.MemoryLocationSet):
            continue
        name = alloc.memorylocations[0].name
        if alloc.kind == "ExternalInput":
            if nc.partition_id_tensor is None or name != nc.partition_id_tensor.name:
                in_names.append(name)
        elif alloc.kind == "ExternalOutput":
            out_names.append(name)
            shape = tuple(alloc.tensor_shape)
            dtype = mybir.dt.np(alloc.dtype)
            out_avals.append(jax.core.ShapedArray(shape, dtype))
            zero_outs.append(np.zeros(shape, dtype))
    n_params = len(in_names)
    n_outs = len(out_avals)
    all_names = in_names + out_names
    if nc.partition_id_tensor is not None:
        all_names = all_names + [nc.partition_id_tensor.name]

    def _body(*args):
        operands = list(args)
        if nc.partition_id_tensor is not None:
            operands.append(bass2jax.partition_id_tensor())
        outs = bass2jax._bass_exec_p.bind(
            *operands,
            out_avals=tuple(out_avals),
            in_names=tuple(all_names),
            out_names=tuple(out_names),
            lowering_input_output_aliases=(),
            sim_require_finite=True,
            sim_require_nnan=True,
            nc=nc,
        )
        return tuple(outs)

    devices = jax.devices()[:NCORES]
    mesh = Mesh(np.asarray(devices), ("core",))
    donate = tuple(range(n_params, n_params + n_outs))
    sharded = jax.jit(
        shard_map(
            _body,
            mesh=mesh,
            in_specs=(PartitionSpec("core"),) * (n_params + n_outs),
            out_specs=(PartitionSpec("core"),) * n_outs,
            check_rep=False,
        ),
        donate_argnums=donate,
        keep_unused=True,
    )
    _STATE["runner"] = (sharded, in_names, out_names, out_avals, zero_outs)
    return _STATE["runner"]


def run_timed(inputs, iters=20):
    """Run via a persistent executable; return (outputs, per_iter_ns)."""
    import time as _time

    import jax

    sharded, in_names, out_names, out_avals, zero_outs = _make_runner()
    in_maps = _prep_in_maps(inputs)
    concat_in = [
        np.concatenate([np.asarray(in_maps[c][n]) for c in range(NCORES)], axis=0)
        for n in in_names
    ]
    dev_in = [jax.device_put(a) for a in concat_in]

    def zeros():
        return [
            np.zeros((NCORES * z.shape[0], *z.shape[1:]), z.dtype) for z in zero_outs
        ]

    # warmup (compiles on first call)
    outs = sharded(*dev_in, *zeros())
    jax.block_until_ready(outs)
    out_np = [np.asarray(o) for o in outs]

    zbufs = [zeros() for _ in range(iters)]
    t0 = _time.perf_counter()
    last = None
    for i in range(iters):
        last = sharded(*dev_in, *zbufs[i])
    jax.block_until_ready(last)
    t1 = _time.perf_counter()
    per_iter_ns = (t1 - t0) / iters * 1e9

    per_core = [
        {
            name: out_np[i].reshape(NCORES, *out_avals[i].shape)[c]
            for i, name in enumerate(out_names)
        }
        for c in range(NCORES)
    ]
    U = np.empty((N,), np.float32)
    F = np.empty((N,), np.float32)
    Ft = np.empty((N,), np.float32)
    for c in range(NCORES):
        o = per_core[c]["out"]
        U[c * NLOC : (c + 1) * NLOC] = o[0]
        F[c * NLOC : (c + 1) * NLOC] = o[1]
        Ft[c * NLOC : (c + 1) * NLOC] = o[2]
    shp = (B, S, 1)
    return (U.reshape(shp), F.reshape(shp), Ft.reshape(shp)), per_iter_ns


# revision 77
# speedup vs baseline: 1.0127x; 1.0127x over previous
"""Trainium2 Bass kernel for the CapacityNN PINN forward pass.

Computes, for N = B*S collocation points x = (s, t):
  U   = MLP([s_norm, t_norm]) * tgt_std + tgt_mean
  F   = U_t  - G(U)             (G = Verhulst logistic growth term)
  F_t = U_tt - G'(U) * U_t
where U_t/U_tt are 1st/2nd derivatives w.r.t. t_norm, computed exactly by
second-order forward-mode Taylor (jet) propagation through the tanh MLP.

Sharding: pure data parallel over 8 NeuronCores (8192 points/core),
MLP weights + PDE scalars replicated. All math runs on-device; the host
only folds weights (O(H^2)) and reorders data (transpose, shard/gather).

Device layout: hidden dim (256) on partitions (2 halves m of 128), points
on the free dim in chunks of 1024; the per-layer stream tiles hold both
halves side by side as [128, 2048] f16 so each elementwise op covers the
whole layer. All matmuls are f16 with f32 PSUM accumulation (1 col/cycle,
same rate as f32r, half the error-prone-ness of the elementwise budget).
Streams: Hv = tanh values; H1 carries sqrt(2)*a1 (sqrt2 folded into the
layer-1 weight copy so st = zc1^2 = 2*z1^2 needs no extra scale; the
final projection's H1 column of W4 is divided by sqrt2); H2 = a2.

Per hidden layer l and chunk (both halves):
  PE : 24 matmuls [128,128]@[128,512] -> pzv/pz1/pz2 ([128,1024] psum)
  Act: tanh(pzv)->av, copy(pz1)->zc1, copy(pz2)->qtc (only Act+DVE may
       touch PSUM on TRN2; GPSIMD/DMA cannot)
  DVE: ee=av*av (l1,l2 on Pool), dm=1-ee (in-place, 4x-mode ts),
       h1=dm*zc1, st=zc1*zc1, tt=av*st, qt=qtc-tt (in-place), h2=dm*qt
       (f16 SBUF ops run at DVE 2x rate, 0.52 ns/col)
  Pool: ee for l1/l2 + half of st(l2) (slow 2 ns/col engine, overflow only)

Schedule: all 8 chunks software-pipelined flat (in-order engine queues
are the schedule): L0 x8, L1 x8, L2 x8, then L3 with the final
projections trailing 4 chunks behind; the PDE tail runs in two halves,
the first overlapped with the second half's compute. PSUM is one
4-deep pool of [128,1024] tiles (8 banks); the final projection
writes rows 0:3 of a pool slot. Weight/scalar loads are batched into
~13 wide DMAs split across the SP and Act queues so the first matmul
starts at ~5 us.
"""

import os
import sys
import tempfile

import numpy as np

for _p in ("/opt/trn_rl_repo", "/root/.axon_site/_ro/trn_rl_repo"):
    if os.path.isdir(_p) and _p not in sys.path:
        sys.path.insert(0, _p)

import concourse.bass as bass
import concourse.bacc as bacc
import concourse.tile as tile
from concourse import mybir
from concourse.bass_utils import run_bass_kernel_spmd

AF = mybir.ActivationFunctionType
OP = mybir.AluOpType
F32 = mybir.dt.float32
F32R = mybir.dt.float32r
F16 = mybir.dt.float16

NCORES = 8
B, S, H = 512, 128, 256
N = B * S                  # 65536 points
NLOC = N // NCORES         # 8192 points per core
CH = 1024                  # points per on-chip chunk
NCHUNK = NLOC // CH
PT = CH // 512             # 512-wide point tiles per chunk
PPP = NLOC // 128          # points per partition in the tail layout (64)
SQRT2 = float(np.sqrt(2.0))


def _build():
    nc = bacc.Bacc(
        "TRN2",
        target_bir_lowering=False,
        debug=False,
        enable_asserts=False,
        num_devices=NCORES,
    )

    def din(name, shape, dt=F32):
        return nc.dram_tensor(name, list(shape), dt, kind="ExternalInput").ap()

    x2 = din("x2", (2, NLOC), F32R)            # rows: raw s, raw t (per-core slice)
    w0t = din("w0t", (2, H), F32R)             # W0.T
    wth = {l: din(f"w{l}th", (H, H), F16) for l in (1, 2, 3)}  # fp16 Wl.T
    w1wt = din("w1wt", (H, H), F16)     # sqrt2*(W1*diag(w0c1)).T fp16
    w1w2t = din("w1w2t", (H, H), F16)   # (-2*W1*diag(w0c1^2)).T fp16
    lt4h = din("lt4h", (6, 128, 3), F16)   # block-diag final lhsT (scaled)
    ball = din("ball", (H, 4))          # biases b0..b3 as columns
    aux = din("aux", (10,))             # packed scalars (see _prep_in_maps)
    out = nc.dram_tensor("out", [3, NLOC], F32, kind="ExternalOutput").ap()

    with tile.TileContext(nc) as tc:
        from contextlib import ExitStack

        with ExitStack() as ctx:
            const = ctx.enter_context(tc.tile_pool(name="const", bufs=1))
            sb = ctx.enter_context(tc.tile_pool(name="sb", bufs=1))
            ps = ctx.enter_context(tc.tile_pool(name="ps", bufs=1, space="PSUM"))

            # ---------- one-time prep (DMAs batched; L0-critical first) ------
            # w0t arrives pre-scaled by 1/(in_std+eps); bias col 0 of ball is
            # the host-folded beta0 = b0 - W0_scaled @ in_mean.
            w0ts = const.tile([2, H], F32R, name="w0ts")
            nc.sync.dma_start(out=w0ts, in_=w0t)

            # aux = [m0, m1, s0, s1, lgr, lcc, lil, tgt_mean, tgt_std, b4]
            auxt = const.tile([128, 10], F32, name="auxt")
            nc.sync.dma_start(
                out=auxt, in_=bass.AP(aux.tensor, 0, [[0, 128], [1, 10]])
            )
            bc_m0 = auxt[:, 0:1]
            bc_m1 = auxt[:, 1:2]
            bc_s0 = auxt[:, 2:3]
            bc_s1 = auxt[:, 3:4]
            bc_lgr = auxt[:, 4:5]
            bc_lcc = auxt[:, 5:6]
            bc_lil = auxt[:, 6:7]
            bc_tm = auxt[:, 7:8]
            bc_ts = auxt[:, 8:9]
            bc_b4 = auxt[:, 9:10]

            def new1(name):
                return const.tile([128, 1], F32, name=name)

            r_t = new1("r_t")
            nc.scalar.activation(r_t, bc_lgr, AF.Exp, 0.0, -1.0)   # exp(-lgr)
            K_t = new1("K_t")
            nc.scalar.activation(K_t, bc_lcc, AF.Sigmoid)
            nc.vector.tensor_scalar(K_t, K_t, 0.8, 0.2, OP.mult, OP.add)
            C_t = new1("C_t")
            nc.scalar.activation(C_t, bc_lil, AF.Sigmoid)
            nc.vector.tensor_scalar(C_t, C_t, 0.1, None, OP.mult)
            ikc = new1("ikc")                                      # 1/(K-C)
            nc.vector.tensor_tensor(ikc, K_t, C_t, OP.subtract)
            nc.vector.reciprocal(ikc, ikc)
            nr = new1("nr")                                        # -r
            nc.vector.tensor_scalar(nr, r_t, -1.0, None, OP.mult)
            c1 = new1("c1")                                        # -1/(K-C)
            nc.vector.tensor_scalar(c1, ikc, -1.0, None, OP.mult)
            mc3 = new1("mc3")                                      # 2r/(K-C)
            nc.vector.tensor_tensor(mc3, r_t, ikc, OP.mult)
            nc.vector.tensor_scalar(mc3, mc3, 2.0, None, OP.mult)
            tmb = new1("tmb")                                      # b4*ts + tm
            nc.vector.tensor_tensor(tmb, bc_b4, bc_ts, OP.mult)
            nc.vector.tensor_tensor(tmb, tmb, bc_tm, OP.add)

            # biases: [128, 4] per m-half (cols = layers 0..3)
            ballt = []
            for m in range(2):
                t = const.tile([128, 4], F32, name=f"ball_{m}")
                nc.sync.dma_start(
                    out=t, in_=bass.AP(ball.tensor, 128 * m * 4, [[4, 128], [1, 4]])
                )
                ballt.append(t)
            bl = {l: [ballt[m][:, l : l + 1] for m in range(2)] for l in (1, 2, 3)}

            beta0 = [ballt[m][:, 0:1] for m in range(2)]

            # ---------- hidden-layer weights (f16, [128,256] per k-half) -----
            def load_w(srcd, nm):
                halves = []
                for kk in range(2):
                    t = const.tile([128, H], F16, name=f"{nm}_{kk}")
                    nc.sync.dma_start(
                        out=t,
                        in_=bass.AP(srcd.tensor, kk * 128 * H, [[H, 128], [1, H]]),
                    )
                    halves.append(t)
                return [
                    [halves[kk][:, mm * 128 : (mm + 1) * 128] for mm in range(2)]
                    for kk in range(2)
                ]

            wtw = load_w(w1wt, "wtw")
            wtw2 = load_w(w1w2t, "wtw2")
            wt16 = {1: load_w(wth[1], "wth1")}

            # final-projection block-diag lhsT: one [128, 6x3] load
            lt4t = const.tile([128, 6, 3], F16, name="lt4t")
            nc.sync.dma_start(
                out=lt4t, in_=bass.AP(lt4h.tensor, 0, [[3, 128], [384, 6], [1, 3]])
            )
            lt4 = [
                [lt4t[:, 2 * s_idx + kk, :] for kk in range(2)] for s_idx in range(3)
            ]

            # the rest of the weights stream in behind the first chunks' L0/L1
            for l in (2, 3):
                wt16[l] = load_w(wth[l], f"wth{l}")

            # ---------- main loop over point chunks ----------
            # Two chunks are processed in lock-step (software pipelining):
            # stage order L0(c0) L0(c1) L1(c0) L1(c1) ... FIN(c0) FIN(c1), so
            # each in-order engine queue works on chunk c0's stage while the
            # producers of chunk c1's stage run on other engines.
            y3f = sb.tile([3, NLOC], F16, name="y3f")
            state = {}   # chunk -> (Hv, H1, H2)

            # streams/transients hold both 128-row halves side by side:
            # [128, 2*CH], half m occupying cols [m*CH : (m+1)*CH].
            def emit_l0(c):
                p = c % 8
                q = c % 4
                x2c = sb.tile([2, CH], F32R, tag=f"x2c{c % 2}", bufs=2)
                dma_eng = nc.scalar if c < 4 else nc.sync
                dma_eng.dma_start(out=x2c, in_=x2[:, c * CH : (c + 1) * CH])
                av = sb.tile([128, 2 * CH], F16, tag=f"hv{p}", bufs=1, name="av0")
                for m in range(2):
                    pz0 = ps.tile([128, CH], F32, tag="pz", bufs=4, name="pz0")
                    for i in range(PT):
                        nc.tensor.matmul(
                            pz0[:, i * 512 : (i + 1) * 512],
                            w0ts[:, m * 128 : (m + 1) * 128],
                            x2c[:, i * 512 : (i + 1) * 512],
                            start=True,
                            stop=True,
                        )
                    nc.scalar.activation(
                        av[:, m * CH : (m + 1) * CH], pz0, AF.Tanh, beta0[m]
                    )
                ee = sb.tile([128, 2 * CH], F16, tag=f"ee{q}", bufs=1, name="ee0")
                nc.vector.tensor_tensor(ee, av, av, OP.mult)
                dm = sb.tile([128, 2 * CH], F16, tag=f"h1_{p}", bufs=1, name="dm0")
                nc.vector.tensor_scalar(dm, ee, -1.0, 1.0, OP.mult, OP.add)
                ad = sb.tile([128, 2 * CH], F16, tag=f"h2_{p}", bufs=1, name="ad0")
                nc.vector.tensor_tensor(ad, av, dm, OP.mult)
                state[c] = (av, dm, ad)

            def emit_layer(c, l):
                p = c % 8
                q = c % 4
                Hv, H1, H2 = state[c]
                w_v = wt16[l]
                w_1 = wtw if l == 1 else wt16[l]
                w_2 = wtw2 if l == 1 else wt16[l]
                av = sb.tile([128, 2 * CH], F16, tag=f"hv{p}", bufs=1, name="av")
                zc1 = sb.tile([128, 2 * CH], F16, tag=f"zc1{q}", bufs=1, name="zc1")
                qtc = sb.tile([128, 2 * CH], F16, tag=f"zc2{q}", bufs=1, name="qtc")
                for m in range(2):
                    # --- primal + first-tangent matmuls, Act psum readers ---
                    pzv = ps.tile([128, CH], F32, tag="pz", bufs=4, name="pzv")
                    for i in range(PT):
                        for kk in range(2):
                            nc.tensor.matmul(
                                pzv[:, i * 512 : (i + 1) * 512],
                                w_v[kk][m],
                                Hv[:, kk * CH + i * 512 : kk * CH + (i + 1) * 512],
                                start=(kk == 0),
                                stop=(kk == 1),
                            )
                    nc.scalar.activation(
                        av[:, m * CH : (m + 1) * CH], pzv, AF.Tanh, bl[l][m]
                    )

                    pz1 = ps.tile([128, CH], F32, tag="pz", bufs=4, name="pz1")
                    for i in range(PT):
                        for kk in range(2):
                            nc.tensor.matmul(
                                pz1[:, i * 512 : (i + 1) * 512],
                                w_1[kk][m],
                                H1[:, kk * CH + i * 512 : kk * CH + (i + 1) * 512],
                                start=(kk == 0),
                                stop=(kk == 1),
                            )
                    nc.scalar.copy(zc1[:, m * CH : (m + 1) * CH], pz1)  # sqrt2*z1

                # --- second-tangent matmuls ---
                for m in range(2):
                    pz2 = ps.tile([128, CH], F32, tag="pz", bufs=4, name="pz2")
                    for i in range(PT):
                        for kk in range(2):
                            nc.tensor.matmul(
                                pz2[:, i * 512 : (i + 1) * 512],
                                w_2[kk][m],
                                H2[:, kk * CH + i * 512 : kk * CH + (i + 1) * 512],
                                start=(kk == 0),
                                stop=(kk == 1),
                            )
                    nc.scalar.copy(qtc[:, m * CH : (m + 1) * CH], pz2)  # z2

                # --- f16 SBUF jet algebra, both halves per op ---
                ee = sb.tile([128, 2 * CH], F16, tag=f"ee{q}", bufs=1, name="ee")
                if l in (1, 2):
                    nc.gpsimd.tensor_tensor(ee, av, av, OP.mult)
                else:
                    nc.vector.tensor_tensor(ee, av, av, OP.mult)
                dm = ee  # in-place: dm = 1 - ee
                nc.vector.tensor_scalar(dm, ee, -1.0, 1.0, OP.mult, OP.add)
                h1t = sb.tile([128, 2 * CH], F16, tag=f"h1_{p}", bufs=1, name="h1t")
                nc.vector.tensor_tensor(h1t, dm, zc1, OP.mult)
                st = sb.tile([128, 2 * CH], F16, tag=f"st{q}", bufs=1, name="st")
                if l == 2:
                    # split halves across DVE/Pool to shed DVE columns
                    nc.vector.tensor_tensor(
                        st[:, 0:CH], zc1[:, 0:CH], zc1[:, 0:CH], OP.mult
                    )
                    nc.gpsimd.tensor_tensor(
                        st[:, CH:], zc1[:, CH:], zc1[:, CH:], OP.mult
                    )
                else:
                    nc.vector.tensor_tensor(st, zc1, zc1, OP.mult)  # 2*z1^2
                ttp = st  # in-place: 2a*z1^2
                nc.vector.tensor_tensor(ttp, av, st, OP.mult)
                qt = qtc  # in-place: z2 - 2a*z1^2
                nc.vector.tensor_tensor(qt, qtc, ttp, OP.subtract)
                h2t = sb.tile([128, 2 * CH], F16, tag=f"h2_{p}", bufs=1, name="h2t")
                nc.vector.tensor_tensor(h2t, dm, qt, OP.mult)
                state[c] = (av, h1t, h2t)

            def emit_fin(c):
                Hv, H1, H2 = state[c]
                py = ps.tile([128, CH], F32, tag="pz", bufs=4, name="py")
                for i in range(PT):
                    first = True
                    for s_idx, stream in enumerate((Hv, H1, H2)):
                        for kk in range(2):
                            nc.tensor.matmul(
                                py[0:3, i * 512 : (i + 1) * 512],
                                lt4[s_idx][kk],
                                stream[:, kk * CH + i * 512 : kk * CH + (i + 1) * 512],
                                start=first,
                                stop=(s_idx == 2 and kk == 1),
                            )
                            first = False
                nc.scalar.copy(y3f[:, c * CH : (c + 1) * CH], py[0:3, :])

            NH = NLOC // 2
            PPH = NH // 128

            def emit_tail(h):
                # PDE algebra for half h of the points (reshaped to [128, PPH])
                tp = sb.tile([128, 3 * PPH], F16, tag=f"tp{h % 2}", bufs=2, name="tp")
                dma_q = nc.sync if h < 1 else nc.scalar
                for s_idx in range(3):
                    dma_q.dma_start(
                        out=tp[:, s_idx * PPH : (s_idx + 1) * PPH],
                        in_=y3f[s_idx : s_idx + 1, h * NH : (h + 1) * NH],
                    )
                yv = tp[:, 0:PPH]
                yt = tp[:, PPH : 2 * PPH]
                ytt = tp[:, 2 * PPH : 3 * PPH]
                oc = sb.tile([128, 3 * PPH], F32, tag=f"oc{h % 2}", bufs=2, name="oc")
                U = oc[:, 0:PPH]
                Fo = oc[:, PPH : 2 * PPH]
                Ft = oc[:, 2 * PPH : 3 * PPH]

                def tl(nm):
                    return sb.tile([128, PPH], F32, tag=f"{nm}{h % 2}", bufs=2, name=nm)

                ut = tl("ut")
                utt = tl("utt")
                vv = tl("vv")
                v2 = tl("v2")
                w1 = tl("w1")
                q1 = tl("q1")
                t1 = tl("t1")
                nc.vector.tensor_scalar(U, yv, bc_ts, tmb, OP.mult, OP.add)
                nc.vector.tensor_scalar(ut, yt, bc_ts, None, OP.mult)
                nc.vector.tensor_scalar(utt, ytt, bc_ts, None, OP.mult)
                nc.vector.tensor_scalar(vv, U, C_t, None, OP.subtract)
                nc.gpsimd.tensor_tensor(v2, vv, vv, OP.mult)
                nc.vector.scalar_tensor_tensor(w1, v2, c1, vv, OP.mult, OP.add)
                nc.vector.scalar_tensor_tensor(Fo, w1, nr, ut, OP.mult, OP.add)
                nc.gpsimd.tensor_tensor(q1, vv, ut, OP.mult)
                nc.vector.scalar_tensor_tensor(t1, ut, nr, utt, OP.mult, OP.add)
                nc.vector.scalar_tensor_tensor(Ft, q1, mc3, t1, OP.mult, OP.add)
                for s_idx in range(3):
                    dma_q.dma_start(
                        out=out[s_idx : s_idx + 1, h * NH : (h + 1) * NH],
                        in_=oc[:, s_idx * PPH : (s_idx + 1) * PPH],
                    )

            for c in range(NCHUNK):
                emit_l0(c)
            for l in (1, 2):
                for c in range(NCHUNK):
                    emit_layer(c, l)
            pend = []
            for c in range(NCHUNK):
                emit_layer(c, 3)
                pend.append(c)
                if len(pend) > 4:
                    fc = pend.pop(0)
                    emit_fin(fc)
                    if fc == 3:
                        emit_tail(0)
            for fc in pend:
                emit_fin(fc)
                if fc == 3:
                    emit_tail(0)
            emit_tail(1)


    nc.compile()
    return nc


_STATE = {}


def _get_nc():
    if "nc" not in _STATE:
        _STATE["nc"] = _build()
    return _STATE["nc"]


def _make_lt4(w4):
    scales = (1.0, 1.0 / SQRT2, 1.0)
    out = np.zeros((6, 128, 3), np.float32)
    for s_idx in range(3):
        for kk in range(2):
            out[2 * s_idx + kk, :, s_idx] = (
                w4[0, kk * 128 : (kk + 1) * 128] * scales[s_idx]
            )
    return out


def _prep_in_maps(inputs):
    f = np.float32

    def arr(k):
        return np.ascontiguousarray(np.asarray(inputs[k], f))

    x = np.asarray(inputs["inputs"], f).reshape(N, 2)
    aux = np.concatenate([
        arr("in_mean"), arr("in_std"),
        arr("log_growth_rate").reshape(1),
        arr("log_carrying_capacity").reshape(1),
        arr("log_initial_loss").reshape(1),
        arr("tgt_mean"), arr("tgt_std"), arr("b4").reshape(1),
    ]).astype(f)
    w0s = arr("W0") / (arr("in_std")[None, :] + np.float32(1e-8))
    beta0 = arr("b0") - w0s @ arr("in_mean")
    ball = np.stack(
        [beta0] + [arr(f"b{l}") for l in (1, 2, 3)], axis=1
    ).astype(f)
    shared = {
        "w0t": np.ascontiguousarray(
            arr("W0").T / (arr("in_std")[:, None] + np.float32(1e-8))
        ),
        "lt4h": _make_lt4(arr("W4").reshape(1, H)).astype(np.float16),
        "w1th": np.ascontiguousarray(arr("W1").T).astype(np.float16),
        "w1wt": np.ascontiguousarray(
            (SQRT2 * arr("W1") * arr("W0")[:, 1][None, :]).T
        ).astype(np.float16),
        "w1w2t": np.ascontiguousarray(
            (arr("W1") * (-2.0 * arr("W0")[:, 1] ** 2)[None, :]).T
        ).astype(np.float16),
        "w2th": np.ascontiguousarray(arr("W2").T).astype(np.float16),
        "w3th": np.ascontiguousarray(arr("W3").T).astype(np.float16),
        "ball": np.ascontiguousarray(ball),
        "aux": aux,
    }
    in_maps = []
    for c in range(NCORES):
        m = dict(shared)
        m["x2"] = np.ascontiguousarray(x[c * NLOC : (c + 1) * NLOC].T)
        in_maps.append(m)
    return in_maps


def run(inputs, trace=False):
    nc = _get_nc()
    in_maps = _prep_in_maps(inputs)
    kw = {}
    if trace:
        kw["tmpdir"] = tempfile.mkdtemp(prefix="bassk_prof_")
    res = run_bass_kernel_spmd(
        nc, in_maps, core_ids=list(range(NCORES)), trace=trace, **kw
    )
    U = np.empty((N,), np.float32)
    F = np.empty((N,), np.float32)
    Ft = np.empty((N,), np.float32)
    for c in range(NCORES):
        o = res.results[c]["out"]
        U[c * NLOC : (c + 1) * NLOC] = o[0]
        F[c * NLOC : (c + 1) * NLOC] = o[1]
        Ft[c * NLOC : (c + 1) * NLOC] = o[2]
    shp = (B, S, 1)
    return (U.reshape(shp), F.reshape(shp), Ft.reshape(shp)), res


def kernel(**inputs):
    outs, _ = run(inputs, trace=False)
    return outs


# ---------------------------------------------------------------------------
# Dev-loop timing: persistent jitted executable (mirrors
# bass2jax.run_bass_via_pjrt's multi-core branch) so repeated executions
# reuse one compiled NEFF and can be timed back-to-back.
# ---------------------------------------------------------------------------
def _make_runner():
    if "runner" in _STATE:
        return _STATE["runner"]
    import jax
    from jax.experimental.shard_map import shard_map
    from jax.sharding import Mesh, PartitionSpec
    from concourse import bass2jax

    bass2jax.install_neuronx_cc_hook()
    nc = _get_nc()

    in_names, out_names, out_avals, zero_outs = [], [], [], []
    for alloc in nc.m.functions[0].allocations:
        if not isinstance(alloc, mybir.MemoryLocationSet):
            continue
        name = alloc.memorylocations[0].name
        if alloc.kind == "ExternalInput":
            if nc.partition_id_tensor is None or name != nc.partition_id_tensor.name:
                in_names.append(name)
        elif alloc.kind == "ExternalOutput":
            out_names.append(name)
            shape = tuple(alloc.tensor_shape)
            dtype = mybir.dt.np(alloc.dtype)
            out_avals.append(jax.core.ShapedArray(shape, dtype))
            zero_outs.append(np.zeros(shape, dtype))
    n_params = len(in_names)
    n_outs = len(out_avals)
    all_names = in_names + out_names
    if nc.partition_id_tensor is not None:
        all_names = all_names + [nc.partition_id_tensor.name]

    def _body(*args):
        operands = list(args)
        if nc.partition_id_tensor is not None:
            operands.append(bass2jax.partition_id_tensor())
        outs = bass2jax._bass_exec_p.bind(
            *operands,
            out_avals=tuple(out_avals),
            in_names=tuple(all_names),
            out_names=tuple(out_names),
            lowering_input_output_aliases=(),
            sim_require_finite=True,
            sim_require_nnan=True,
            nc=nc,
        )
        return tuple(outs)

    devices = jax.devices()[:NCORES]
    mesh = Mesh(np.asarray(devices), ("core",))
    donate = tuple(range(n_params, n_params + n_outs))
    sharded = jax.jit(
        shard_map(
            _body,
            mesh=mesh,
            in_specs=(PartitionSpec("core"),) * (n_params + n_outs),
            out_specs=(PartitionSpec("core"),) * n_outs,
            check_rep=False,
        ),
        donate_argnums=donate,
        keep_unused=True,
    )
    _STATE["runner"] = (sharded, in_names, out_names, out_avals, zero_outs)
    return _STATE["runner"]


def run_timed(inputs, iters=20):
    """Run via a persistent executable; return (outputs, per_iter_ns)."""
    import time as _time

    import jax

    sharded, in_names, out_names, out_avals, zero_outs = _make_runner()
    in_maps = _prep_in_maps(inputs)
    concat_in = [
        np.concatenate([np.asarray(in_maps[c][n]) for c in range(NCORES)], axis=0)
        for n in in_names
    ]
    dev_in = [jax.device_put(a) for a in concat_in]

    def zeros():
        return [
            np.zeros((NCORES * z.shape[0], *z.shape[1:]), z.dtype) for z in zero_outs
        ]

    # warmup (compiles on first call)
    outs = sharded(*dev_in, *zeros())
    jax.block_until_ready(outs)
    out_np = [np.asarray(o) for o in outs]

    zbufs = [zeros() for _ in range(iters)]
    t0 = _time.perf_counter()
    last = None
    for i in range(iters):
        last = sharded(*dev_in, *zbufs[i])
    jax.block_until_ready(last)
    t1 = _time.perf_counter()
    per_iter_ns = (t1 - t0) / iters * 1e9

    per_core = [
        {
            name: out_np[i].reshape(NCORES, *out_avals[i].shape)[c]
            for i, name in enumerate(out_names)
        }
        for c in range(NCORES)
    ]
    U = np.empty((N,), np.float32)
    F = np.empty((N,), np.float32)
    Ft = np.empty((N,), np.float32)
    for c in range(NCORES):
        o = per_core[c]["out"]
        U[c * NLOC : (c + 1) * NLOC] = o[0]
        F[c * NLOC : (c + 1) * NLOC] = o[1]
        Ft[c * NLOC : (c + 1) * NLOC] = o[2]
    shp = (B, S, 1)
    return (U.reshape(shp), F.reshape(shp), Ft.reshape(shp)), per_iter_ns
